# revision 4
# baseline (speedup 1.0000x reference)
"""Multi-head attention TRN2 kernel, head-parallel across 8 NeuronCores.

Per core c (= head h=c), all matmuls in float32r (11-bit mantissa, full PE
rate at N=512), keys-on-partitions score layout (no on-device transposes):

  QT[e,s] = Wq_h^T q^T        (lhsT = Wq_h[d,e] nat, rhs = qT[d,s])
  KT[e,t], V[t,e] likewise
  scoresT[t,s] = K~ Q~^T      (lhsT = KT[e,t], rhs = QT[e,s])
  E = exp(scoresT*scale + wbias[t]) masked multiplicatively via additive
      -1e9 mask folded pre-exp (DVE scalar_tensor_tensor)
  rowsum (broadcast to 128 partitions) via ones[128,128] lhsT matmul
  OT[e,s] = V^T E / rowsum    (lhsT = V[t,e], rhs = E[t,s])
  out[s,:] = sum_e OT[e,s]^T Wo_h[e,:]

Host: transposes q/k/v, pre-rounds f32r inputs (RNE drop-12, bit-exact vs
HW cast), builds mask^T additive bf16 mask, folds all biases exactly
(bk drops under softmax; bq -> per-key exp bias; bv,bo -> final add),
sums the 8 per-head partial outputs.
"""
import sys
import numpy as np

sys.path.insert(0, "/opt/trn_rl_repo")

H, D, B, S = 8, 512, 2, 2048
P = 128
NE = D // P            # 4 e/d tiles
NT = S // P            # 16 key tiles per batch
CH = 512               # query-chunk width
NCH = S // CH          # 4 chunks per batch
SCALE = 1.0 / np.sqrt(np.float32(D))

_CACHE = {}


def _f32r_round(x):
    """Bit-exact host emulation of HW fp32->f32r cast (RNE, drop 12 bits)."""
    u = np.ascontiguousarray(x, np.float32).view(np.uint32).astype(np.uint64)
    half = np.uint64(1 << 11)
    lsb = (u >> np.uint64(12)) & np.uint64(1)
    u2 = (u + half - np.uint64(1) + lsb) >> np.uint64(12) << np.uint64(12)
    return u2.astype(np.uint32).view(np.float32).reshape(x.shape)


def _build():
    from contextlib import ExitStack
    from concourse import bass, bacc, tile

    mybir = bass.mybir
    dt = mybir.dt
    AF = mybir.ActivationFunctionType
    ALU = mybir.AluOpType

    nc = bacc.Bacc("TRN2", target_bir_lowering=False, debug=False)

    qT_d = nc.dram_tensor("qT", [D, B * S], dt.float32r, kind="ExternalInput")
    kT_d = nc.dram_tensor("kT", [D, B * S], dt.float32r, kind="ExternalInput")
    vT_d = nc.dram_tensor("vT", [D, B * S], dt.float32r, kind="ExternalInput")
    mT_d = nc.dram_tensor("mT", [S, S], dt.bfloat16, kind="ExternalInput")
    wq_d = nc.dram_tensor("wq", [D, D], dt.float32r, kind="ExternalInput")
    wk_d = nc.dram_tensor("wk", [D, D], dt.float32r, kind="ExternalInput")
    wv_d = nc.dram_tensor("wv", [D, D], dt.float32r, kind="ExternalInput")
    wo_d = nc.dram_tensor("wo", [D, D], dt.float32r, kind="ExternalInput")
    wb_d = nc.dram_tensor("wb", [P, B * NT], dt.float32, kind="ExternalInput")
    out_d = nc.dram_tensor("out", [B * S, D], dt.float32, kind="ExternalOutput")

    # [D, X] row-major -> [128, D/128, X] partition-major view
    def dtiles(ap_2d, ncols):
        return ap_2d.rearrange("(a p) c -> p a c", p=P)

    with tile.TileContext(nc) as tc:
        with ExitStack() as ctx:
            wpool = ctx.enter_context(tc.tile_pool(name="w", bufs=1))
            kvpool = ctx.enter_context(tc.tile_pool(name="kv", bufs=1))
            xin = ctx.enter_context(tc.tile_pool(name="xin", bufs=3))
            epool = ctx.enter_context(tc.tile_pool(name="e", bufs=1))
            mpool = ctx.enter_context(tc.tile_pool(name="m", bufs=1))
            qtpool = ctx.enter_context(tc.tile_pool(name="qt", bufs=1))
            otpool = ctx.enter_context(tc.tile_pool(name="ot", bufs=1))
            tpool = ctx.enter_context(tc.tile_pool(name="tmp", bufs=3))
            rpool = ctx.enter_context(tc.tile_pool(name="r", bufs=2))
            opool = ctx.enter_context(tc.tile_pool(name="o", bufs=2))
            psA = ctx.enter_context(tc.tile_pool(name="psA", bufs=4, space="PSUM"))
            psO = ctx.enter_context(tc.tile_pool(name="psO", bufs=2, space="PSUM"))
            psF = ctx.enter_context(tc.tile_pool(name="psF", bufs=2, space="PSUM"))

            # weights resident
            wq = wpool.tile([P, NE, D], dt.float32r)
            wk = wpool.tile([P, NE, D], dt.float32r)
            wv = wpool.tile([P, NE, D], dt.float32r)
            wo = wpool.tile([P, NE, D], dt.float32r)
            for t, d in ((wq, wq_d), (wk, wk_d), (wv, wv_d), (wo, wo_d)):
                nc.sync.dma_start(t[:], dtiles(d.ap(), D))
            wb = wpool.tile([P, B * NT], dt.float32)
            nc.sync.dma_start(wb[:], wb_d[:])
            onesf = wpool.tile([P, P], dt.float32)
            nc.vector.memset(onesf[:], 1.0)
            ones = wpool.tile([P, P], dt.float32r)
            nc.vector.tensor_copy(ones[:], onesf[:])

            KT = kvpool.tile([P, NE, S], dt.float32r, tag="KT")
            V = kvpool.tile([P, NT, CH], dt.float32r, tag="V")  # [t, e]: NT tiles x D cols
            # note: V free layout is [NT, D] with D == CH == 512

            qTt = dtiles(qT_d.ap(), B * S)
            kTt = dtiles(kT_d.ap(), B * S)
            vTt = dtiles(vT_d.ap(), B * S)
            mTt = mT_d.ap().rearrange("(a p) c -> p a c", p=P)  # [128, NT, S]

            for b in range(B):
                # ---- stage A: project K^T and V for this batch ----
                for tc4 in range(NCH):
                    col0 = b * S + tc4 * CH
                    kin = xin.tile([P, NE, CH], dt.float32r, tag="xin")
                    nc.sync.dma_start(kin[:], kTt[:, :, col0:col0 + CH])
                    vin = xin.tile([P, NE, CH], dt.float32r, tag="xin")
                    nc.sync.dma_start(vin[:], vTt[:, :, col0:col0 + CH])
                    for et in range(NE):
                        ps = psA.tile([P, CH], dt.float32)
                        for kd in range(NE):
                            nc.tensor.matmul(
                                ps[:], wk[:, kd, et * P:(et + 1) * P], kin[:, kd, :],
                                start=(kd == 0), stop=(kd == NE - 1))
                        nc.vector.tensor_copy(KT[:, et, tc4 * CH:(tc4 + 1) * CH], ps[:])
                    for ts in range(CH // P):
                        ps = psA.tile([P, D], dt.float32)
                        for kd in range(NE):
                            nc.tensor.matmul(
                                ps[:], vin[:, kd, ts * P:(ts + 1) * P], wv[:, kd, :],
                                start=(kd == 0), stop=(kd == NE - 1))
                        nc.vector.tensor_copy(V[:, tc4 * (CH // P) + ts, :], ps[:])

                # ---- stage B: per query-chunk attention ----
                for c in range(NCH):
                    col0 = b * S + c * CH
                    qin = xin.tile([P, NE, CH], dt.float32r, tag="xin")
                    nc.sync.dma_start(qin[:], qTt[:, :, col0:col0 + CH])
                    mt = mpool.tile([P, NT, CH], dt.bfloat16)
                    nc.sync.dma_start(mt[:], mTt[:, :, c * CH:(c + 1) * CH])

                    QT = qtpool.tile([P, NE, CH], dt.float32r)
                    for et in range(NE):
                        ps = psA.tile([P, CH], dt.float32)
                        for kd in range(NE):
                            nc.tensor.matmul(
                                ps[:], wq[:, kd, et * P:(et + 1) * P], qin[:, kd, :],
                                start=(kd == 0), stop=(kd == NE - 1))
                        nc.vector.tensor_copy(QT[:, et, :], ps[:])

                    E = epool.tile([P, NT, CH], dt.float32r)
                    for tt in range(NT):
                        ps = psA.tile([P, CH], dt.float32)
                        for et in range(NE):
                            nc.tensor.matmul(
                                ps[:], KT[:, et, tt * P:(tt + 1) * P], QT[:, et, :],
                                start=(et == 0), stop=(et == NE - 1))
                        tmp = tpool.tile([P, CH], dt.float32)
                        nc.vector.scalar_tensor_tensor(
                            tmp[:], ps[:], float(SCALE), mt[:, tt, :],
                            op0=ALU.mult, op1=ALU.add)
                        nc.scalar.activation(
                            E[:, tt, :], tmp[:], AF.Exp,
                            bias=wb[:, b * NT + tt: b * NT + tt + 1], scale=1.0)

                    psr = psA.tile([P, CH], dt.float32, tag="ps")
                    for tt in range(NT):
                        nc.tensor.matmul(ps_r_dummy := psr[:], ones[:], E[:, tt, :],
                                         start=(tt == 0), stop=(tt == NT - 1))
                    recip = rpool.tile([P, CH], dt.float32)
                    nc.vector.reciprocal(recip[:], psr[:])

                    OT = otpool.tile([P, NE, CH], dt.float32r)
                    for es in range(NE):
                        ps = psO.tile([P, CH], dt.float32)
                        for tt in range(NT):
                            nc.tensor.matmul(
                                ps[:], V[:, tt, es * P:(es + 1) * P], E[:, tt, :],
                                start=(tt == 0), stop=(tt == NT - 1))
                        nc.vector.tensor_mul(OT[:, es, :], ps[:], recip[:])

                    for ss in range(CH // P):
                        ps = psF.tile([P, D], dt.float32)
                        for et in range(NE):
                            nc.tensor.matmul(
                                ps[:], OT[:, et, ss * P:(ss + 1) * P], wo[:, et, :],
                                start=(et == 0), stop=(et == NE - 1))
                        ot = opool.tile([P, D], dt.float32)
                        nc.vector.tensor_copy(ot[:], ps[:])
                        r0 = col0 + ss * P
                        nc.sync.dma_start(out_d[r0:r0 + P, :], ot[:])

    nc.compile()
    return nc


def kernel(q, k, v, mask, Wq, bq, Wk, bk, Wv, bv, Wo, bo):
    from concourse.bass_utils import run_bass_kernel_spmd
    import ml_dtypes

    q = np.asarray(q, np.float32)
    k = np.asarray(k, np.float32)
    v = np.asarray(v, np.float32)
    mask = np.asarray(mask)
    Wq = np.asarray(Wq, np.float32)
    Wk = np.asarray(Wk, np.float32)
    Wv = np.asarray(Wv, np.float32)
    Wo = np.asarray(Wo, np.float32)
    bq = np.asarray(bq, np.float32)
    bk = np.asarray(bk, np.float32)
    bv = np.asarray(bv, np.float32)
    bo = np.asarray(bo, np.float32)

    # host prep
    qT = _f32r_round(q.transpose(2, 0, 1).reshape(D, B * S))
    kT = _f32r_round(k.transpose(2, 0, 1).reshape(D, B * S))
    vT = _f32r_round(v.transpose(2, 0, 1).reshape(D, B * S))
    # additive mask, transposed: [t, s], 0 where allowed, -1e9 where blocked
    mT = np.where(mask.T == 1, np.float32(-1e9), np.float32(0.0)).astype(ml_dtypes.bfloat16)
    mT = np.ascontiguousarray(mT)

    in_maps = []
    for h in range(H):
        # per-key exp bias: bq . K~_t / sqrt(D) = k_t . (Wk_h bq_h) / sqrt(D)
        wvec = (k.reshape(B * S, D) @ (Wk[h] @ bq[h])) * SCALE  # [B*S]
        wb = np.ascontiguousarray(wvec.reshape(B * NT, P).T.astype(np.float32))
        in_maps.append({
            "qT": qT, "kT": kT, "vT": vT, "mT": mT,
            "wq": _f32r_round(Wq[h]),
            "wk": _f32r_round(Wk[h]),
            "wv": _f32r_round(Wv[h]),
            "wo": _f32r_round(Wo[h * D:(h + 1) * D, :]),
            "wb": wb,
        })

    if "nc" not in _CACHE:
        _CACHE["nc"] = _build()
    nc = _CACHE["nc"]
    _CACHE["in_maps"] = in_maps

    res = run_bass_kernel_spmd(nc, in_maps, core_ids=list(range(H)))
    parts = np.stack([res.results[h]["out"] for h in range(H)])  # [H, B*S, D]
    total = parts.sum(axis=0, dtype=np.float64)

    # exact bias folds: sum_h bv_h @ Wo_h + bo  (attention rows sum to 1)
    cvec = bo.astype(np.float64).copy()
    for h in range(H):
        cvec += bv[h].astype(np.float64) @ Wo[h * D:(h + 1) * D, :].astype(np.float64)
    total += cvec
    return total.astype(np.float32).reshape(B, S, D)
